# revision 20
# baseline (speedup 1.0000x reference)
"""MetaPathGNN kernel for 8 Trainium2 NeuronCores.

Computation (only what the reference output needs — h_b/conv0/edge_ab/x_b are
dead code in the reference):
    msg  = x_a[edge_ba[1]]                      # [E, H] gather
    aggr = segment_sum(msg, edge_ba[0], N)      # [N, H]
    h_a  = relu(aggr @ wl1.T + x_a @ (w01+w11).T + (bl1+b01+b11))
    out  = h_a @ out_w.T + out_b

Sharding: destination nodes split into 8 contiguous shards of 6250. Each core
gathers the source rows for its own edges from a full replica of x_a (no
collectives needed), aggregates via one-hot matmuls into PSUM, then applies
the linear layers in feature-major (transposed) layout.

Aggregation scheme per core:
  - edges sorted by destination, bucketed into 128-dest windows
  - 128 edges at a time form a "chunk": gathered rows land as a
    [128 edges, 128 feat] SBUF tile (lhsT), a one-hot S [128 edges, 128 dest]
    is built on DVE with is_equal(dest_id, iota), and
    matmul(psum_bank[:, win*128:(win+1)*128], lhsT=msg, rhs=S) accumulates.
  - PSUM banks hold 512 destinations (4 windows); per bank the first matmul
    uses start=True (pending-zero for the whole bank), everything else
    accumulates.
  - dma_gather indices are int16, so sources are gathered in two passes
    (src < 32768 from base 0, src >= 32768 from base 32768); pad slots use
    idx 0 with dest -1 (all-zero one-hot row -> no contribution).

The SPMD program is shared by all 8 cores, so per-window chunk budgets are
max'ed across cores; pad chunks gather row 0 and contribute nothing.
"""

import numpy as np

P = 8
N = 50000
E = 500000
H = 128
NSH = N // P          # 6250 destinations per core
W = 128               # destination window width (matmul rhs free dim)
GROUP = 512           # PSUM bank width in fp32 columns
NGROUP = (NSH + GROUP - 1) // GROUP   # 13
NWIN = NGROUP * (GROUP // W)          # 52 windows (some pure padding)
SPLIT = 32768         # int16-index limit for dma_gather
GB = 64               # gather batch, in chunks
SBATCH = 8            # chunks per one-hot build op
GATHER_BUFS = 3
S_BUFS = 3
SCRATCH = 65536
CAPS = (28, 28)


def _pack_edges(dst, src):
    """Bucket edges by (core, window, pass) and compute shared chunk budgets.

    Returns (budgets, group_order, CA, CB, per_core): per_core[c] holds int16
    src index arrays and f32 window-local dest arrays for both passes, laid
    out in the processing order (groups sorted heaviest-first).
    """
    core = dst // NSH
    dl = dst - core * NSH
    win = dl // W
    pss = (src >= SPLIT).astype(np.int64)

    counts = np.zeros((P, NWIN, 2), np.int64)
    np.add.at(counts, (core, win, pss), 1)
    chunks_needed = -(-counts // 128)          # ceil
    budgets = chunks_needed.max(axis=0)        # [NWIN, 2]
    budgets[:, 0] = np.maximum(budgets[:, 0], 1)   # every window inits PSUM

    # process heaviest groups first so the post-gather tail is minimal
    gtot = budgets.reshape(NGROUP, 4, 2).sum(axis=(1, 2))
    group_order = list(np.argsort(-gtot, kind="stable"))

    # pad per-pass totals to a multiple of SBATCH (extra pad chunks on the
    # last processed window keep the S-build batching uniform)
    last_w = group_order[-1] * 4 + 3
    for p in range(2):
        tot = int(budgets[:, p].sum())
        pad = (-tot) % SBATCH
        budgets[last_w, p] += pad

    CA = int(budgets[:, 0].sum())
    CB = int(budgets[:, 1].sum())

    # window sequence in processing order -> stream slot offsets per bucket
    wseq = [g * 4 + w4 for g in group_order for w4 in range(4)]
    offs = np.zeros((NWIN, 2), np.int64)
    for p in range(2):
        acc = 0
        for w in wseq:
            offs[w, p] = acc
            acc += int(budgets[w, p]) * 128

    per_core = []
    for c in range(P):
        m = core == c
        dlc, winc, pc, sc = dl[m], win[m], pss[m], src[m]
        # order edges by (pass, window); position within bucket via argsort
        order = np.lexsort((dlc, winc, pc))
        dlc, winc, pc, sc = dlc[order], winc[order], pc[order], sc[order]
        # rank within each (pass, window) bucket
        key = pc * NWIN + winc
        # edges are sorted by key; rank = index - first index of key
        first = np.zeros(2 * NWIN, np.int64)
        cnt = np.bincount(key, minlength=2 * NWIN)
        first[1:] = np.cumsum(cnt)[:-1]
        rank = np.arange(len(key)) - first[key]
        slot = offs[winc, pc] + rank

        arrs = {}
        for p, name in ((0, "A"), (1, "B")):
            L = (CA if p == 0 else CB) * 128
            idx = np.zeros(L, np.int64)          # pad -> row 0 of the pass base
            dest = np.full(L, -1.0, np.float32)  # pad -> no one-hot match
            mm = pc == p
            idx[slot[mm]] = sc[mm] - (SPLIT if p else 0)
            dest[slot[mm]] = (dlc[mm] - winc[mm] * W).astype(np.float32)
            arrs["idx" + name] = idx.astype(np.int16)
            arrs["dest" + name] = dest.reshape(-1, 128).T.copy()  # [slot, chunk]
        per_core.append(arrs)

    return budgets, group_order, CA, CB, per_core


def _wrap_idx(idx):
    """dma_gather index layout: element i at [i % 16, i // 16], tiled to 128
    partitions."""
    w = np.ascontiguousarray(idx.reshape(-1, 16).T)  # [16, L/16]
    return np.tile(w, (8, 1))


def _build_program(budgets, group_order, CA, CB):
    import concourse.bacc as bacc
    import concourse.tile as tile
    import concourse.mybir as mybir

    F32 = mybir.dt.float32
    F16 = mybir.dt.float16
    I16 = mybir.dt.int16
    NCOL = NGROUP * GROUP

    nc = bacc.Bacc("TRN2", num_swdge_queues=4, dynamic_dma_scratch_size=SCRATCH)
    xa_d = nc.dram_tensor("xa", [N, H], F16, kind="ExternalInput")
    xaT_d = nc.dram_tensor("xaT", [H, NCOL], F16, kind="ExternalInput")
    idxA_d = nc.dram_tensor("idxA", [128, CA * 8], I16, kind="ExternalInput")
    idxB_d = nc.dram_tensor("idxB", [128, CB * 8], I16, kind="ExternalInput")
    destA_d = nc.dram_tensor("destA", [128, CA], F32, kind="ExternalInput")
    destB_d = nc.dram_tensor("destB", [128, CB], F32, kind="ExternalInput")
    wagg_d = nc.dram_tensor("wagg", [H, H], F16, kind="ExternalInput")
    wx_d = nc.dram_tensor("wx", [H, H], F16, kind="ExternalInput")
    wo_d = nc.dram_tensor("wo", [H, H], F16, kind="ExternalInput")
    bh_d = nc.dram_tensor("bh", [H, 1], F32, kind="ExternalInput")
    bo_d = nc.dram_tensor("bo", [1, H], F16, kind="ExternalInput")
    ones_d = nc.dram_tensor("ones", [1, GROUP], F16, kind="ExternalInput")
    iota_d = nc.dram_tensor("iota", [128, SBATCH * W], F16, kind="ExternalInput")
    outT_d = nc.dram_tensor("outT", [H, NCOL], F32, kind="ExternalOutput")

    CN = [CA, CB]
    # group-aligned gather batches: cut at group boundaries, capped in size,
    # so the consumption tail after the last gather is at most ~one group
    caps = CAPS
    batches = []   # per pass: list of (start_chunk, nchunks)
    for p in range(2):
        per_group = [
            int(budgets[g * 4:(g + 1) * 4, p].sum()) for g in group_order
        ]
        blist, start, cur = [], 0, 0
        for g, n in enumerate(per_group):
            if cur and cur + n > caps[p]:
                blist.append((start, cur))
                start, cur = start + cur, 0
            cur += n
        if cur:
            blist.append((start, cur))
        batches.append(blist)
    gbmax = max(n for bl in batches for _, n in bl)
    idx_d = [idxA_d, idxB_d]
    dest_d = [destA_d, destB_d]
    base = [(0, SPLIT), (SPLIT, N)]

    with tile.TileContext(nc) as tc:
        with (
            tc.tile_pool(name="const", bufs=1) as constp,
            tc.tile_pool(name="gath", bufs=GATHER_BUFS) as gathp,
            tc.tile_pool(name="gidx", bufs=GATHER_BUFS) as gidxp,
            tc.tile_pool(name="sbld", bufs=S_BUFS) as spool,
            tc.tile_pool(name="post", bufs=2) as postp,
            tc.tile_pool(name="ps", bufs=2, space="PSUM") as psump,
        ):
            dest_t = []
            for p in range(2):
                dt_ = constp.tile([128, CN[p]], F32, tag=f"dest{p}", name=f"dest{p}")
                dest_t.append(dt_)
            for p in range(2):
                nc.sync.dma_start(dest_t[p][:], dest_d[p][:])
            iota_t = constp.tile([128, SBATCH * W], F16)
            nc.sync.dma_start(iota_t[:], iota_d[:])
            wagg_t = constp.tile([H, H], F16, tag="wagg")
            wx_t = constp.tile([H, H], F16, tag="wx")
            wo_t = constp.tile([H, H], F16, tag="wo")
            bh_t = constp.tile([H, 1], F32, tag="bh")
            bo_t = constp.tile([1, H], F16, tag="bo")
            ones_t = constp.tile([1, GROUP], F16, tag="ones")
            for t, dd in ((wagg_t, wagg_d), (wx_t, wx_d), (wo_t, wo_d),
                          (bh_t, bh_d), (bo_t, bo_d), (ones_t, ones_d)):
                nc.sync.dma_start(t[:], dd[:])

            # streaming state per pass: current batch tile / S tile
            cur_batch = [None, None]
            cur_s = [None, None]
            consumed = [0, 0]  # chunks consumed per pass

            batch_pos = [0, 0]   # next batch index per pass
            batch_start = [0, 0]
            qrr = [0]            # SWDGE queue round-robin

            def chunk_tiles(p, c):
                """(lhsT msg AP, rhs S AP) for chunk c of pass p; emits the
                gather / S-build on first touch of their batch."""
                if cur_batch[p] is None or c >= batch_start[p] + cur_batch[p].shape[1]:
                    start, nch = batches[p][batch_pos[p]]
                    assert start == c, (p, c, start)
                    batch_pos[p] += 1
                    batch_start[p] = start
                    it = gidxp.tile([128, gbmax * 8], I16, tag=f"i{p}", name=f"bidx{p}")
                    nc.sync.dma_start(it[:, :nch * 8],
                                      idx_d[p][:, start * 8:(start + nch) * 8])
                    t = gathp.tile([128, gbmax, H], F16, tag=f"g{p}")
                    t = t[:, :nch, :]
                    lo, hi = base[p]
                    nc.gpsimd.dma_gather(
                        t[:],
                        xa_d[lo:hi, :],
                        it[:, :nch * 8],
                        nch * 128,
                        nch * 128,
                        H,
                        single_packet=False,
                        queue_num=qrr[0] % 4,
                    )
                    qrr[0] += 1
                    cur_batch[p] = t
                r = c - batch_start[p]
                sb, sr = divmod(c, SBATCH)
                if sr == 0:
                    st = spool.tile([128, SBATCH, W], F16, tag=f"s{p}")
                    cur_s[p] = st
                # one-hot row for this chunk: S[j, i] = (iota[i] == dest[j])
                nc.vector.tensor_scalar(
                    out=cur_s[p][:, sr, :],
                    in0=iota_t[:, :W],
                    scalar1=dest_t[p][:, c:c + 1],
                    scalar2=None,
                    op0=mybir.AluOpType.is_equal,
                )
                return cur_batch[p][:, r, :], cur_s[p][:, sr, :]

            relu = mybir.ActivationFunctionType.Relu
            copyf = mybir.ActivationFunctionType.Copy

            for g in group_order:
                aggr_ps = psump.tile([128, GROUP], F32, tag="aggr")
                nmm = sum(int(budgets[g * 4 + w4, p]) for w4 in range(4) for p in range(2))
                mmi = 0
                for w4 in range(4):
                    w = g * 4 + w4
                    for p in range(2):
                        for _ in range(int(budgets[w, p])):
                            lhsT, rhs = chunk_tiles(p, consumed[p])
                            consumed[p] += 1
                            nc.tensor.matmul(
                                aggr_ps[:, w4 * W:(w4 + 1) * W], lhsT, rhs,
                                start=(mmi == 0), stop=(mmi == nmm - 1),
                            )
                            mmi += 1
                aggr_sb = postp.tile([128, GROUP], F16, tag="aggr_sb")
                nc.scalar.activation(aggr_sb[:], aggr_ps[:], copyf)
                xaT_t = postp.tile([128, GROUP], F16, tag="xaT", bufs=3, name=f"xaT{g}")
                nc.sync.dma_start(xaT_t[:], xaT_d[:, g * GROUP:(g + 1) * GROUP])
                z_ps = psump.tile([128, GROUP], F32, tag="z")
                nc.tensor.matmul(z_ps[:], wagg_t[:], aggr_sb[:], start=True, stop=False)
                nc.tensor.matmul(z_ps[:], wx_t[:], xaT_t[:],
                                 start=False, stop=True)
                h_sb = postp.tile([128, GROUP], F16, tag="h")
                nc.scalar.activation(h_sb[:], z_ps[:], relu, bias=bh_t[:, 0:1])
                o_ps = psump.tile([128, GROUP], F32, tag="o")
                nc.tensor.matmul(o_ps[:], wo_t[:], h_sb[:], start=True, stop=False)
                nc.tensor.matmul(o_ps[:], bo_t[:], ones_t[:], start=False, stop=True)
                o_sb = postp.tile([128, GROUP], F32, tag="osb")
                nc.scalar.activation(o_sb[:], o_ps[:], copyf)
                nc.sync.dma_start(outT_d[:, g * GROUP:(g + 1) * GROUP], o_sb[:])

    nc.compile()
    return nc


def prepare(inputs):
    """Host-side packing: returns (nc, in_maps)."""
    x_a = np.ascontiguousarray(np.asarray(inputs["x_a"], dtype=np.float32))
    eb = np.asarray(inputs["edge_ba"])
    dst = eb[0].astype(np.int64)
    src = eb[1].astype(np.int64)

    wagg = np.ascontiguousarray(np.asarray(inputs["conv1_wl_w"], np.float32).T.astype(np.float16))
    wx = np.ascontiguousarray(
        (np.asarray(inputs["conv1_w0_w"], np.float32)
         + np.asarray(inputs["conv1_w1_w"], np.float32)).T.astype(np.float16))
    bh = (np.asarray(inputs["conv1_wl_b"], np.float32)
          + np.asarray(inputs["conv1_w0_b"], np.float32)
          + np.asarray(inputs["conv1_w1_b"], np.float32)).reshape(H, 1)
    wo = np.ascontiguousarray(np.asarray(inputs["out_w"], np.float32).T.astype(np.float16))
    bo = np.asarray(inputs["out_b"], np.float32).reshape(1, H).astype(np.float16)
    iota = np.tile(np.arange(W, dtype=np.float16), SBATCH)[None, :].repeat(128, 0)
    iota = np.ascontiguousarray(iota)
    xa16 = x_a.astype(np.float16)

    budgets, group_order, CA, CB, per_core = _pack_edges(dst, src)
    nc = _build_program(budgets, group_order, CA, CB)

    NCOL = NGROUP * GROUP
    in_maps = []
    for c in range(P):
        xaT = np.zeros((H, NCOL), np.float16)
        xaT[:, :NSH] = x_a[c * NSH:(c + 1) * NSH].T.astype(np.float16)
        a = per_core[c]
        in_maps.append({
            "xa": xa16,
            "xaT": xaT,
            "idxA": _wrap_idx(a["idxA"]),
            "idxB": _wrap_idx(a["idxB"]),
            "destA": a["destA"],
            "destB": a["destB"],
            "wagg": wagg, "wx": wx, "wo": wo, "bh": bh, "bo": bo,
            "ones": np.ones((1, GROUP), np.float16),
            "iota": iota,
        })
    return nc, in_maps


def assemble(results):
    out = np.empty((N, H), np.float32)
    for c in range(P):
        out[c * NSH:(c + 1) * NSH] = results[c]["outT"][:, :NSH].T
    return out


def kernel(**inputs):
    from concourse.bass_utils import run_bass_kernel_spmd

    nc, in_maps = prepare(inputs)
    r = run_bass_kernel_spmd(nc, in_maps, list(range(P)))
    return assemble(r.results)


# revision 21
# speedup vs baseline: 1.0111x; 1.0111x over previous
"""MetaPathGNN kernel for 8 Trainium2 NeuronCores.

Computation (only what the reference output needs — h_b/conv0/edge_ab/x_b are
dead code in the reference):
    msg  = x_a[edge_ba[1]]                      # [E, H] gather
    aggr = segment_sum(msg, edge_ba[0], N)      # [N, H]
    h_a  = relu(aggr @ wl1.T + x_a @ (w01+w11).T + (bl1+b01+b11))
    out  = h_a @ out_w.T + out_b

Sharding: destination nodes split into 8 contiguous shards of 6250. Each core
gathers the source rows for its own edges from a full replica of x_a (no
collectives needed), aggregates via one-hot matmuls into PSUM, then applies
the linear layers in feature-major (transposed) layout.

Aggregation scheme per core:
  - edges sorted by destination, bucketed into 128-dest windows
  - 128 edges at a time form a "chunk": gathered rows land as a
    [128 edges, 128 feat] SBUF tile (lhsT), a one-hot S [128 edges, 128 dest]
    is built on DVE with is_equal(dest_id, iota), and
    matmul(psum_bank[:, win*128:(win+1)*128], lhsT=msg, rhs=S) accumulates.
  - PSUM banks hold 512 destinations (4 windows); per bank the first matmul
    uses start=True (pending-zero for the whole bank), everything else
    accumulates.
  - dma_gather indices are int16, so sources are gathered in two passes
    (src < 32768 from base 0, src >= 32768 from base 32768); pad slots use
    idx 0 with dest -1 (all-zero one-hot row -> no contribution).

The SPMD program is shared by all 8 cores, so per-window chunk budgets are
max'ed across cores; pad chunks gather row 0 and contribute nothing.
"""

import numpy as np

P = 8
N = 50000
E = 500000
H = 128
NSH = N // P          # 6250 destinations per core
W = 128               # destination window width (matmul rhs free dim)
GROUP = 512           # PSUM bank width in fp32 columns
NGROUP = (NSH + GROUP - 1) // GROUP   # 13
NWIN = NGROUP * (GROUP // W)          # 52 windows (some pure padding)
SPLIT = 32768         # int16-index limit for dma_gather
GB = 64               # gather batch, in chunks
SBATCH = 8            # chunks per one-hot build op
GATHER_BUFS = 3
S_BUFS = 3
SCRATCH = 98304
CAPS = (20, 20)


def _pack_edges(dst, src):
    """Bucket edges by (core, window, pass) and compute shared chunk budgets.

    Returns (budgets, group_order, CA, CB, per_core): per_core[c] holds int16
    src index arrays and f32 window-local dest arrays for both passes, laid
    out in the processing order (groups sorted heaviest-first).
    """
    core = dst // NSH
    dl = dst - core * NSH
    win = dl // W
    pss = (src >= SPLIT).astype(np.int64)

    counts = np.zeros((P, NWIN, 2), np.int64)
    np.add.at(counts, (core, win, pss), 1)
    chunks_needed = -(-counts // 128)          # ceil
    budgets = chunks_needed.max(axis=0)        # [NWIN, 2]
    budgets[:, 0] = np.maximum(budgets[:, 0], 1)   # every window inits PSUM

    # process heaviest groups first so the post-gather tail is minimal
    gtot = budgets.reshape(NGROUP, 4, 2).sum(axis=(1, 2))
    group_order = list(np.argsort(-gtot, kind="stable"))

    # pad per-pass totals to a multiple of SBATCH (extra pad chunks on the
    # last processed window keep the S-build batching uniform)
    last_w = group_order[-1] * 4 + 3
    for p in range(2):
        tot = int(budgets[:, p].sum())
        pad = (-tot) % SBATCH
        budgets[last_w, p] += pad

    CA = int(budgets[:, 0].sum())
    CB = int(budgets[:, 1].sum())

    # window sequence in processing order -> stream slot offsets per bucket
    wseq = [g * 4 + w4 for g in group_order for w4 in range(4)]
    offs = np.zeros((NWIN, 2), np.int64)
    for p in range(2):
        acc = 0
        for w in wseq:
            offs[w, p] = acc
            acc += int(budgets[w, p]) * 128

    per_core = []
    for c in range(P):
        m = core == c
        dlc, winc, pc, sc = dl[m], win[m], pss[m], src[m]
        # order edges by (pass, window); position within bucket via argsort
        order = np.lexsort((dlc, winc, pc))
        dlc, winc, pc, sc = dlc[order], winc[order], pc[order], sc[order]
        # rank within each (pass, window) bucket
        key = pc * NWIN + winc
        # edges are sorted by key; rank = index - first index of key
        first = np.zeros(2 * NWIN, np.int64)
        cnt = np.bincount(key, minlength=2 * NWIN)
        first[1:] = np.cumsum(cnt)[:-1]
        rank = np.arange(len(key)) - first[key]
        slot = offs[winc, pc] + rank

        arrs = {}
        for p, name in ((0, "A"), (1, "B")):
            L = (CA if p == 0 else CB) * 128
            idx = np.zeros(L, np.int64)          # pad -> row 0 of the pass base
            dest = np.full(L, -1.0, np.float32)  # pad -> no one-hot match
            mm = pc == p
            idx[slot[mm]] = sc[mm] - (SPLIT if p else 0)
            dest[slot[mm]] = (dlc[mm] - winc[mm] * W).astype(np.float32)
            arrs["idx" + name] = idx.astype(np.int16)
            arrs["dest" + name] = dest.reshape(-1, 128).T.copy()  # [slot, chunk]
        per_core.append(arrs)

    return budgets, group_order, CA, CB, per_core


def _wrap_idx(idx):
    """dma_gather index layout: element i at [i % 16, i // 16], tiled to 128
    partitions."""
    w = np.ascontiguousarray(idx.reshape(-1, 16).T)  # [16, L/16]
    return np.tile(w, (8, 1))


def _build_program(budgets, group_order, CA, CB):
    import concourse.bacc as bacc
    import concourse.tile as tile
    import concourse.mybir as mybir

    F32 = mybir.dt.float32
    F16 = mybir.dt.float16
    I16 = mybir.dt.int16
    NCOL = NGROUP * GROUP

    nc = bacc.Bacc("TRN2", num_swdge_queues=4, dynamic_dma_scratch_size=SCRATCH)
    xa_d = nc.dram_tensor("xa", [N, H], F16, kind="ExternalInput")
    xaT_d = nc.dram_tensor("xaT", [H, NCOL], F16, kind="ExternalInput")
    idxA_d = nc.dram_tensor("idxA", [128, CA * 8], I16, kind="ExternalInput")
    idxB_d = nc.dram_tensor("idxB", [128, CB * 8], I16, kind="ExternalInput")
    destA_d = nc.dram_tensor("destA", [128, CA], F32, kind="ExternalInput")
    destB_d = nc.dram_tensor("destB", [128, CB], F32, kind="ExternalInput")
    wagg_d = nc.dram_tensor("wagg", [H, H], F16, kind="ExternalInput")
    wx_d = nc.dram_tensor("wx", [H, H], F16, kind="ExternalInput")
    wo_d = nc.dram_tensor("wo", [H, H], F16, kind="ExternalInput")
    bh_d = nc.dram_tensor("bh", [H, 1], F32, kind="ExternalInput")
    bo_d = nc.dram_tensor("bo", [1, H], F16, kind="ExternalInput")
    ones_d = nc.dram_tensor("ones", [1, GROUP], F16, kind="ExternalInput")
    iota_d = nc.dram_tensor("iota", [128, SBATCH * W], F16, kind="ExternalInput")
    outT_d = nc.dram_tensor("outT", [H, NCOL], F32, kind="ExternalOutput")

    CN = [CA, CB]
    # group-aligned gather batches: cut at group boundaries, capped in size,
    # so the consumption tail after the last gather is at most ~one group
    caps = CAPS
    batches = []   # per pass: list of (start_chunk, nchunks)
    for p in range(2):
        per_group = [
            int(budgets[g * 4:(g + 1) * 4, p].sum()) for g in group_order
        ]
        blist, start, cur = [], 0, 0
        for g, n in enumerate(per_group):
            if cur and cur + n > caps[p]:
                blist.append((start, cur))
                start, cur = start + cur, 0
            cur += n
        if cur:
            blist.append((start, cur))
        batches.append(blist)
    gbmax = max(n for bl in batches for _, n in bl)
    idx_d = [idxA_d, idxB_d]
    dest_d = [destA_d, destB_d]
    base = [(0, SPLIT), (SPLIT, N)]

    with tile.TileContext(nc) as tc:
        with (
            tc.tile_pool(name="const", bufs=1) as constp,
            tc.tile_pool(name="gath", bufs=GATHER_BUFS) as gathp,
            tc.tile_pool(name="gidx", bufs=GATHER_BUFS) as gidxp,
            tc.tile_pool(name="sbld", bufs=S_BUFS) as spool,
            tc.tile_pool(name="post", bufs=2) as postp,
            tc.tile_pool(name="ps", bufs=2, space="PSUM") as psump,
        ):
            dest_t = []
            for p in range(2):
                dt_ = constp.tile([128, CN[p]], F32, tag=f"dest{p}", name=f"dest{p}")
                dest_t.append(dt_)
            for p in range(2):
                nc.sync.dma_start(dest_t[p][:], dest_d[p][:])
            iota_t = constp.tile([128, SBATCH * W], F16)
            nc.sync.dma_start(iota_t[:], iota_d[:])
            wagg_t = constp.tile([H, H], F16, tag="wagg")
            wx_t = constp.tile([H, H], F16, tag="wx")
            wo_t = constp.tile([H, H], F16, tag="wo")
            bh_t = constp.tile([H, 1], F32, tag="bh")
            bo_t = constp.tile([1, H], F16, tag="bo")
            ones_t = constp.tile([1, GROUP], F16, tag="ones")
            for t, dd in ((wagg_t, wagg_d), (wx_t, wx_d), (wo_t, wo_d),
                          (bh_t, bh_d), (bo_t, bo_d), (ones_t, ones_d)):
                nc.sync.dma_start(t[:], dd[:])

            # streaming state per pass: current batch tile / S tile
            cur_batch = [None, None]
            cur_s = [None, None]
            consumed = [0, 0]  # chunks consumed per pass

            batch_pos = [0, 0]   # next batch index per pass
            batch_start = [0, 0]
            qrr = [0]            # SWDGE queue round-robin

            def chunk_tiles(p, c):
                """(lhsT msg AP, rhs S AP) for chunk c of pass p; emits the
                gather / S-build on first touch of their batch."""
                if cur_batch[p] is None or c >= batch_start[p] + cur_batch[p].shape[1]:
                    start, nch = batches[p][batch_pos[p]]
                    assert start == c, (p, c, start)
                    batch_pos[p] += 1
                    batch_start[p] = start
                    it = gidxp.tile([128, gbmax * 8], I16, tag=f"i{p}", name=f"bidx{p}")
                    nc.sync.dma_start(it[:, :nch * 8],
                                      idx_d[p][:, start * 8:(start + nch) * 8])
                    t = gathp.tile([128, gbmax, H], F16, tag=f"g{p}")
                    t = t[:, :nch, :]
                    lo, hi = base[p]
                    nc.gpsimd.dma_gather(
                        t[:],
                        xa_d[lo:hi, :],
                        it[:, :nch * 8],
                        nch * 128,
                        nch * 128,
                        H,
                        single_packet=False,
                        queue_num=qrr[0] % 4,
                    )
                    qrr[0] += 1
                    cur_batch[p] = t
                r = c - batch_start[p]
                sb, sr = divmod(c, SBATCH)
                if sr == 0:
                    st = spool.tile([128, SBATCH, W], F16, tag=f"s{p}")
                    cur_s[p] = st
                # one-hot row for this chunk: S[j, i] = (iota[i] == dest[j])
                nc.vector.tensor_scalar(
                    out=cur_s[p][:, sr, :],
                    in0=iota_t[:, :W],
                    scalar1=dest_t[p][:, c:c + 1],
                    scalar2=None,
                    op0=mybir.AluOpType.is_equal,
                )
                return cur_batch[p][:, r, :], cur_s[p][:, sr, :]

            relu = mybir.ActivationFunctionType.Relu
            copyf = mybir.ActivationFunctionType.Copy

            for g in group_order:
                aggr_ps = psump.tile([128, GROUP], F32, tag="aggr")
                nmm = sum(int(budgets[g * 4 + w4, p]) for w4 in range(4) for p in range(2))
                mmi = 0
                for w4 in range(4):
                    w = g * 4 + w4
                    for p in range(2):
                        for _ in range(int(budgets[w, p])):
                            lhsT, rhs = chunk_tiles(p, consumed[p])
                            consumed[p] += 1
                            nc.tensor.matmul(
                                aggr_ps[:, w4 * W:(w4 + 1) * W], lhsT, rhs,
                                start=(mmi == 0), stop=(mmi == nmm - 1),
                            )
                            mmi += 1
                aggr_sb = postp.tile([128, GROUP], F16, tag="aggr_sb")
                nc.scalar.activation(aggr_sb[:], aggr_ps[:], copyf)
                xaT_t = postp.tile([128, GROUP], F16, tag="xaT", bufs=3, name=f"xaT{g}")
                nc.sync.dma_start(xaT_t[:], xaT_d[:, g * GROUP:(g + 1) * GROUP])
                z_ps = psump.tile([128, GROUP], F32, tag="z")
                nc.tensor.matmul(z_ps[:], wagg_t[:], aggr_sb[:], start=True, stop=False)
                nc.tensor.matmul(z_ps[:], wx_t[:], xaT_t[:],
                                 start=False, stop=True)
                h_sb = postp.tile([128, GROUP], F16, tag="h")
                nc.scalar.activation(h_sb[:], z_ps[:], relu, bias=bh_t[:, 0:1])
                o_ps = psump.tile([128, GROUP], F32, tag="o")
                nc.tensor.matmul(o_ps[:], wo_t[:], h_sb[:], start=True, stop=False)
                nc.tensor.matmul(o_ps[:], bo_t[:], ones_t[:], start=False, stop=True)
                o_sb = postp.tile([128, GROUP], F32, tag="osb")
                nc.scalar.activation(o_sb[:], o_ps[:], copyf)
                nc.sync.dma_start(outT_d[:, g * GROUP:(g + 1) * GROUP], o_sb[:])

    nc.compile()
    return nc


def prepare(inputs):
    """Host-side packing: returns (nc, in_maps)."""
    x_a = np.ascontiguousarray(np.asarray(inputs["x_a"], dtype=np.float32))
    eb = np.asarray(inputs["edge_ba"])
    dst = eb[0].astype(np.int64)
    src = eb[1].astype(np.int64)

    wagg = np.ascontiguousarray(np.asarray(inputs["conv1_wl_w"], np.float32).T.astype(np.float16))
    wx = np.ascontiguousarray(
        (np.asarray(inputs["conv1_w0_w"], np.float32)
         + np.asarray(inputs["conv1_w1_w"], np.float32)).T.astype(np.float16))
    bh = (np.asarray(inputs["conv1_wl_b"], np.float32)
          + np.asarray(inputs["conv1_w0_b"], np.float32)
          + np.asarray(inputs["conv1_w1_b"], np.float32)).reshape(H, 1)
    wo = np.ascontiguousarray(np.asarray(inputs["out_w"], np.float32).T.astype(np.float16))
    bo = np.asarray(inputs["out_b"], np.float32).reshape(1, H).astype(np.float16)
    iota = np.tile(np.arange(W, dtype=np.float16), SBATCH)[None, :].repeat(128, 0)
    iota = np.ascontiguousarray(iota)
    xa16 = x_a.astype(np.float16)

    budgets, group_order, CA, CB, per_core = _pack_edges(dst, src)
    nc = _build_program(budgets, group_order, CA, CB)

    NCOL = NGROUP * GROUP
    in_maps = []
    for c in range(P):
        xaT = np.zeros((H, NCOL), np.float16)
        xaT[:, :NSH] = x_a[c * NSH:(c + 1) * NSH].T.astype(np.float16)
        a = per_core[c]
        in_maps.append({
            "xa": xa16,
            "xaT": xaT,
            "idxA": _wrap_idx(a["idxA"]),
            "idxB": _wrap_idx(a["idxB"]),
            "destA": a["destA"],
            "destB": a["destB"],
            "wagg": wagg, "wx": wx, "wo": wo, "bh": bh, "bo": bo,
            "ones": np.ones((1, GROUP), np.float16),
            "iota": iota,
        })
    return nc, in_maps


def assemble(results):
    out = np.empty((N, H), np.float32)
    for c in range(P):
        out[c * NSH:(c + 1) * NSH] = results[c]["outT"][:, :NSH].T
    return out


def kernel(**inputs):
    from concourse.bass_utils import run_bass_kernel_spmd

    nc, in_maps = prepare(inputs)
    r = run_bass_kernel_spmd(nc, in_maps, list(range(P)))
    return assemble(r.results)
